# revision 2
# baseline (speedup 1.0000x reference)
"""Trainium2 Bass kernel for windowed local attention (8x8 windows).

Full computation (reference):
  x [B=8, C=192, H=256, W=256] -> window partition (8x8) -> per-window:
  qkv = w_qkv @ win + b_qkv ; attn = softmax(q^T k / sqrt(C)) ;
  out = v @ attn^T ; y = w_proj @ out + b_proj -> window reverse.

Sharding: data-parallel over batch. Core b handles image b (32 window-rows
("bands") of 32 windows each). Weights replicated.

Per-band pipeline (band = [C, 8, W] slab, 4 groups of 8 windows):
  A: q,k = Wq/Wk @ x         (band matmuls, C=192 contraction as 128+64)
  B: v_T = x^T-style matmul producing [tokens, C] directly (no transpose)
  C: scores for window PAIRS (2x64 tokens = 128 partitions, block matmul;
     off-diagonal cross-window blocks are garbage)
  softmax: exp on ACT (no max subtraction: scores ~ N(0,1), |s| < ~7
     over all samples, exp is safe in fp32); garbage blocks zeroed by
     GPSIMD memsets; row-sum + reciprocal + scale on DVE
  D: attn^T via identity matmul on the PE
  E: out = v_T^T @ attn_T    (pair-blocked, zeros kill cross terms)
  F: proj band matmul + bias, permuted copy into band buffer, DMA out.

Bias handling: q,k biases are added during the PSUM->SBUF copy
(ACT Identity activation with per-partition bias). The v bias is folded
into the proj bias on the host: since softmax rows sum to 1,
out = (v0 + bv) @ attn^T = v0 @ attn^T + bv, so
b_proj' = b_proj + w_proj @ bv.  The qk scale is folded into Wq, bq.
"""

import os
import sys

import numpy as np

if "/opt/trn_rl_repo" not in sys.path:
    sys.path.insert(0, "/opt/trn_rl_repo")

C = 192
WS = 8
S = WS * WS  # 64 tokens per window
F32 = None  # set after imports


def build_program(n_bands=32, width=256):
    import concourse.bass as bass  # noqa: F401
    import concourse.tile as tile
    from concourse import bacc, mybir

    f32 = mybir.dt.float32
    GPB = width // 64  # groups per band (8 windows each)

    nc = bacc.Bacc("TRN2", target_bir_lowering=False, debug=False)

    Hn = n_bands * WS
    x = nc.dram_tensor("x", [C, Hn, width], f32, kind="ExternalInput").ap()
    y = nc.dram_tensor("y", [C, Hn, width], f32, kind="ExternalOutput").ap()
    wqT = nc.dram_tensor("wqT", [C, C], f32, kind="ExternalInput").ap()
    wkT = nc.dram_tensor("wkT", [C, C], f32, kind="ExternalInput").ap()
    wvT = nc.dram_tensor("wvT", [C, C], f32, kind="ExternalInput").ap()
    wpT = nc.dram_tensor("wpT", [C, C], f32, kind="ExternalInput").ap()
    bq = nc.dram_tensor("bq", [C, 1], f32, kind="ExternalInput").ap()
    bk = nc.dram_tensor("bk", [C, 1], f32, kind="ExternalInput").ap()
    bpp = nc.dram_tensor("bpp", [C, 1], f32, kind="ExternalInput").ap()
    eye = nc.dram_tensor("eye", [128, 128], f32, kind="ExternalInput").ap()

    Ident = mybir.ActivationFunctionType.Identity
    Exp = mybir.ActivationFunctionType.Exp
    AX = mybir.AxisListType.X

    def blk(t2d, p):
        # [P, 512] -> [P, 128] block p
        return t2d.rearrange("p (pr n) -> p pr n", pr=4)[:, p]

    from contextlib import ExitStack

    with tile.TileContext(nc) as tc, ExitStack() as ctx:
        cp = ctx.enter_context(tc.tile_pool(name="consts", bufs=1))
        xp = ctx.enter_context(tc.tile_pool(name="xbands", bufs=2))
        qkp = ctx.enter_context(tc.tile_pool(name="qk", bufs=2))
        vbp = ctx.enter_context(tc.tile_pool(name="vb", bufs=1))
        vtsp = ctx.enter_context(tc.tile_pool(name="vts", bufs=4))
        ep = ctx.enter_context(tc.tile_pool(name="e", bufs=2))
        atsp = ctx.enter_context(tc.tile_pool(name="ats", bufs=2))
        rp = ctx.enter_context(tc.tile_pool(name="r", bufs=2))
        obp = ctx.enter_context(tc.tile_pool(name="ob", bufs=2))
        fbp = ctx.enter_context(tc.tile_pool(name="fb", bufs=2))
        pp_big = ctx.enter_context(tc.tile_pool(name="pp_big", bufs=4, space="PSUM"))
        pp_vt = ctx.enter_context(tc.tile_pool(name="pp_vt", bufs=2, space="PSUM"))
        pp_sc = ctx.enter_context(tc.tile_pool(name="pp_sc", bufs=1, space="PSUM"))
        pp_at = ctx.enter_context(tc.tile_pool(name="pp_at", bufs=1, space="PSUM"))

        # ---- constants ----
        def const_2d(name, src, p0, p1, cols):
            t = cp.tile([p1 - p0, cols], f32, tag=name)
            nc.sync.dma_start(out=t[:], in_=src[p0:p1, 0:cols])
            return t

        wq1 = const_2d("wq1", wqT, 0, 128, C)
        wq2 = const_2d("wq2", wqT, 128, 192, C)
        wk1 = const_2d("wk1", wkT, 0, 128, C)
        wk2 = const_2d("wk2", wkT, 128, 192, C)
        wv1 = const_2d("wv1", wvT, 0, 128, C)
        wv2 = const_2d("wv2", wvT, 128, 192, C)
        wp1 = const_2d("wp1", wpT, 0, 128, C)
        wp2 = const_2d("wp2", wpT, 128, 192, C)
        bq1 = const_2d("bq1", bq, 0, 128, 1)
        bq2 = const_2d("bq2", bq, 128, 192, 1)
        bk1 = const_2d("bk1", bk, 0, 128, 1)
        bk2 = const_2d("bk2", bk, 128, 192, 1)
        bp1 = const_2d("bp1", bpp, 0, 128, 1)
        bp2 = const_2d("bp2", bpp, 128, 192, 1)
        ident = const_2d("ident", eye, 0, 128, 128)

        for hw in range(n_bands):
            xb1 = xp.tile([128, 8, width], f32, tag="xb1")
            nc.sync.dma_start(out=xb1[:], in_=x[0:128, hw * 8:(hw + 1) * 8, :])
            xb2 = xp.tile([64, 8, width], f32, tag="xb2")
            nc.sync.dma_start(out=xb2[:], in_=x[128:192, hw * 8:(hw + 1) * 8, :])
            fb1 = fbp.tile([128, 8, width], f32, tag="fb1")
            fb2 = fbp.tile([64, 8, width], f32, tag="fb2")

            xf1 = xb1[:].rearrange("p i w -> p (i w)")
            xf2 = xb2[:].rearrange("p i w -> p (i w)")

            # ---- A: q, k, v band matmuls (contiguous 512-col chunks = 2
            # rows of the band), copied back into window-major token order ----
            NW = width // WS  # windows per band
            q1 = qkp.tile([128, NW * 64], f32, tag="q1")
            q2 = qkp.tile([64, NW * 64], f32, tag="q2")
            k1 = qkp.tile([128, NW * 64], f32, tag="k1")
            k2 = qkp.tile([64, NW * 64], f32, tag="k2")
            v1 = vbp.tile([128, NW * 64], f32, tag="v1")
            v2 = vbp.tile([64, NW * 64], f32, tag="v2")
            # window-major view, sliced per copy chunk: (i2, ww, j) iteration
            wmv = {
                id(t): t[:].rearrange("p (ww i j) -> p i ww j",
                                      ww=NW, i=8, j=8)
                for t in (q1, q2, k1, k2, v1, v2)
            }
            for ncnk in range(width * 8 // 512):
                q1p = pp_big.tile([128, 512], f32, tag="big")
                q2p = pp_big.tile([64, 512], f32, tag="big")
                k1p = pp_big.tile([128, 512], f32, tag="big")
                k2p = pp_big.tile([64, 512], f32, tag="big")
                v1p = pp_vt.tile([128, 512], f32, tag="vt")
                v2p = pp_vt.tile([64, 512], f32, tag="vt")
                rhs1 = xf1[:, ncnk * 512:(ncnk + 1) * 512]
                rhs2 = xf2[:, ncnk * 512:(ncnk + 1) * 512]
                for w1, w2, op1, op2 in ((wq1, wq2, q1p, q2p),
                                         (wk1, wk2, k1p, k2p),
                                         (wv1, wv2, v1p, v2p)):
                    for mlo, mhi, op in ((0, 128, op1), (128, 192, op2)):
                        nc.tensor.matmul(op[:], w1[:, mlo:mhi], rhs1,
                                         start=True, stop=False)
                        nc.tensor.matmul(op[:], w2[:, mlo:mhi], rhs2,
                                         start=False, stop=True)
                ri2 = 512 // width  # band rows per chunk
                for ps, sb, bias in ((q1p, q1, bq1), (q2p, q2, bq2),
                                     (k1p, k1, bk1), (k2p, k2, bk2)):
                    outv = wmv[id(sb)][:, ncnk * ri2:(ncnk + 1) * ri2]
                    nc.scalar.activation(outv, ps[:], Ident, bias=bias[:, 0:1])
                for ps, sb in ((v1p, v1), (v2p, v2)):
                    outv = wmv[id(sb)][:, ncnk * ri2:(ncnk + 1) * ri2]
                    nc.vector.tensor_copy(outv, ps[:])

            for g in range(GPB):
                # ---- B: v_T per pair via identity matmul (PE transpose) ----
                vts_half = []
                for half in range(2):
                    vtp = pp_vt.tile([128, 384], f32, tag="vt")
                    for pi in range(2):
                        p = 2 * half + pi
                        col = pi * 192
                        off = (g * 8 + 2 * p) * 64
                        nc.tensor.matmul(vtp[:, col:col + 128],
                                         v1[:, off:off + 128], ident[:])
                        nc.tensor.matmul(vtp[:, col + 128:col + 192],
                                         v2[:, off:off + 128],
                                         ident[0:64, 0:64])
                    vts = vtsp.tile([128, 384], f32, tag="vts")
                    nc.vector.tensor_copy(vts[:], vtp[:])
                    vts_half.append(vts)

                # ---- C: pair-blocked scores ----
                scp = pp_sc.tile([128, 512], f32, tag="sc")
                for p in range(4):
                    off = (g * 8 + 2 * p) * 64
                    nc.tensor.matmul(blk(scp[:], p), q1[:, off:off + 128],
                                     k1[:, off:off + 128],
                                     start=True, stop=False)
                    nc.tensor.matmul(blk(scp[:], p), q2[:, off:off + 128],
                                     k2[:, off:off + 128],
                                     start=False, stop=True)

                # ---- softmax (no max subtraction; see module docstring) ----
                e = ep.tile([128, 4, 128], f32, tag="e")
                nc.scalar.activation(e[:], scp[:], Exp)
                for p in range(4):
                    nc.gpsimd.memset(e[0:64, p, 64:128], 0.0)
                    nc.gpsimd.memset(e[64:128, p, 0:64], 0.0)
                rs = rp.tile([128, 4], f32, tag="rs")
                nc.vector.reduce_sum(rs[:], e[:], axis=AX)
                ri = rp.tile([128, 4], f32, tag="ri")
                nc.vector.reciprocal(ri[:], rs[:])
                for p in range(4):
                    nc.vector.tensor_scalar_mul(e[:, p, :], e[:, p, :],
                                                ri[:, p:p + 1])

                # ---- D: attn^T via identity matmul ----
                atp = pp_at.tile([128, 4, 128], f32, tag="at")
                for p in range(4):
                    nc.tensor.matmul(atp[:, p], e[:, p, :], ident[:])
                ats = atsp.tile([128, 4, 128], f32, tag="ats")
                nc.vector.tensor_copy(ats[:], atp[:])

                # ---- E: out = v_T^T @ attn_T ----
                eo1 = pp_big.tile([128, 512], f32, tag="big")
                eo2 = pp_big.tile([64, 512], f32, tag="big")
                for p in range(4):
                    vts = vts_half[p // 2]
                    col = (p % 2) * 192
                    nc.tensor.matmul(blk(eo1[:], p), vts[:, col:col + 128],
                                     ats[:, p, :])
                    nc.tensor.matmul(blk(eo2[:], p), vts[:, col + 128:col + 192],
                                     ats[:, p, :])
                ob1 = obp.tile([128, 512], f32, tag="ob1")
                ob2 = obp.tile([64, 512], f32, tag="ob2")
                nc.vector.tensor_copy(ob1[:], eo1[:])
                nc.vector.tensor_copy(ob2[:], eo2[:])

                # ---- F: proj + bias, permuted copy into band buffer ----
                f1 = pp_big.tile([128, 512], f32, tag="big")
                f2 = pp_big.tile([64, 512], f32, tag="big")
                for mlo, mhi, fps in ((0, 128, f1), (128, 192, f2)):
                    nc.tensor.matmul(fps[:], wp1[:, mlo:mhi], ob1[:],
                                     start=True, stop=False)
                    nc.tensor.matmul(fps[:], wp2[:, mlo:mhi], ob2[:],
                                     start=False, stop=True)
                fr1 = fb1[:].rearrange(
                    "p i (gg w8 j) -> p gg w8 i j", gg=GPB, w8=8, j=8)
                fr2 = fb2[:].rearrange(
                    "p i (gg w8 j) -> p gg w8 i j", gg=GPB, w8=8, j=8)
                nc.scalar.activation(fr1[:, g], f1[:], Ident, bias=bp1[:, 0:1])
                nc.scalar.activation(fr2[:, g], f2[:], Ident, bias=bp2[:, 0:1])

            nc.sync.dma_start(out=y[0:128, hw * 8:(hw + 1) * 8, :], in_=fb1[:])
            nc.sync.dma_start(out=y[128:192, hw * 8:(hw + 1) * 8, :], in_=fb2[:])

    nc.compile()
    return nc


def prep_weights(w_qkv, b_qkv, w_proj, b_proj):
    scale = np.float32(C ** -0.5)
    w_qkv = np.asarray(w_qkv, dtype=np.float32)
    b_qkv = np.asarray(b_qkv, dtype=np.float32)
    w_proj = np.asarray(w_proj, dtype=np.float32)
    b_proj = np.asarray(b_proj, dtype=np.float32)
    wq, wk, wv = w_qkv[0:C], w_qkv[C:2 * C], w_qkv[2 * C:3 * C]
    return {
        "wqT": np.ascontiguousarray((wq * scale).T),
        "wkT": np.ascontiguousarray(wk.T),
        "wvT": np.ascontiguousarray(wv.T),
        "wpT": np.ascontiguousarray(w_proj.T),
        "bq": np.ascontiguousarray((b_qkv[0:C] * scale).reshape(C, 1)),
        "bk": np.ascontiguousarray(b_qkv[C:2 * C].reshape(C, 1)),
        "bpp": np.ascontiguousarray(
            (b_proj + w_proj @ b_qkv[2 * C:3 * C]).reshape(C, 1)),
        "eye": np.eye(128, dtype=np.float32),
    }


_PROGRAM_CACHE = {}


def get_program(n_bands, width=256):
    key = (n_bands, width)
    if key not in _PROGRAM_CACHE:
        _PROGRAM_CACHE[key] = build_program(n_bands, width)
    return _PROGRAM_CACHE[key]


def make_in_maps(x, w_qkv, b_qkv, w_proj, b_proj):
    x = np.asarray(x, dtype=np.float32)
    wts = prep_weights(w_qkv, b_qkv, w_proj, b_proj)
    return [{"x": np.ascontiguousarray(x[b]), **wts} for b in range(x.shape[0])]


def assemble_output(results):
    out = np.stack([results[b]["y"] for b in range(len(results))], axis=0)
    return out.astype(np.float32)


def kernel(x, w_qkv, b_qkv, w_proj, b_proj):
    from concourse.bass_utils import run_bass_kernel_spmd

    x = np.asarray(x, dtype=np.float32)
    B, c, H, W = x.shape
    assert c == C
    nc = get_program(H // WS, W)
    in_maps = make_in_maps(x, w_qkv, b_qkv, w_proj, b_proj)
    res = run_bass_kernel_spmd(nc, in_maps, core_ids=list(range(B)))
    return assemble_output(res.results)

